# revision 3
# baseline (speedup 1.0000x reference)
"""Multi-head attention block (LN -> QKV -> attention -> out-proj) on 8 TRN2 cores.

Sharding: (batch, query-half). Core i handles batch i//2, query rows
half (i%2) of 2048. Each core computes LN + K/V for its whole batch,
Q only for its query half, attention, and the out-projection for its
query rows. Output row blocks are disjoint -> no collectives; the host
concatenates.

All 8 cores run ONE graph: the host rolls x by -1024 rows for odd cores
so "my" query rows are always rows 0:1024 (attention is invariant to
K/V row permutation; LN is per-row).

Compute dtype bf16 (PSUM accumulation f32). Host folds ln_gamma and the
softmax scale into w_qkv, and adds beta@w_qkv / b_out on the host
(exact f32).

Attention layout: dots^T = K @ Q^T via lhsT=K^T [64,128], rhs=Q^T
[64,512] -> PSUM [128 k, 512 q]; exp on ScalarE writes attn^T bf16
which is directly the rhs for out^T = V_aug.T @ attn^T with
lhsT=V_aug [128 k, 65] (65th column of ones accumulates the softmax
denominator for free).
"""

import os
import sys

sys.path.insert(0, "/opt/trn_rl_repo")

import ml_dtypes
import numpy as np

import concourse.bass as bass
import concourse.tile as tile
from concourse import bacc, mybir
from concourse.bass_utils import run_bass_kernel_spmd
from concourse.masks import make_identity

F32 = mybir.dt.float32
BF16 = mybir.dt.bfloat16
AF = mybir.ActivationFunctionType

B, N, DIM = 4, 2048, 1024
HEADS, DH = 16, 64
INNER = HEADS * DH  # 1024
SCALE = DH ** -0.5
NQ = N // 2          # query rows per core
N_CORES = 8
RT = N // 128        # 16 row tiles
KT = DIM // 128      # 8 contraction tiles
PAIRS = HEADS // 2   # 8 head pairs
EPS = 1e-5


def _build_graph():
    nc = bacc.Bacc("TRN2", target_bir_lowering=False, debug=False,
                   num_devices=N_CORES)
    x_d = nc.dram_tensor("x", [N, DIM], F32, kind="ExternalInput").ap()
    wqkv_d = nc.dram_tensor("wqkv", [DIM, 3 * INNER], BF16,
                            kind="ExternalInput").ap()
    wout_d = nc.dram_tensor("wout", [INNER, DIM], BF16,
                            kind="ExternalInput").ap()
    out_d = nc.dram_tensor("out", [NQ, DIM], F32, kind="ExternalOutput").ap()

    with tile.TileContext(nc) as tc:
        _kernel_body(tc, x_d, wqkv_d, wout_d, out_d)
    nc.compile()
    return nc


def _kernel_body(tc, x_d, wqkv_d, wout_d, out_d):
    nc = tc.nc
    from contextlib import ExitStack

    with ExitStack() as outer:
        # ---- persistent tiles (live through attention/out-proj) ----
        const_pool = outer.enter_context(tc.tile_pool(name="const", bufs=1))
        qkvp = outer.enter_context(tc.tile_pool(name="qkvp", bufs=1))
        outp = outer.enter_context(tc.tile_pool(name="outp", bufs=1))

        ident = const_pool.tile([128, 128], BF16, tag="ident")
        make_identity(nc, ident[:])
        ones_col = const_pool.tile([1, DH], F32, tag="onescol")
        nc.gpsimd.memset(ones_col[:], 1.0)
        eps_t = const_pool.tile([128, 1], F32, tag="eps")
        nc.gpsimd.memset(eps_t[:], EPS)

        # Q^T [pair*64 x 2][pair-local q], K^T, V(+ones) and out^T stacks
        qt = qkvp.tile([128, PAIRS * NQ], BF16, tag="qt")      # [2h*64, p*1024+q]
        kt_sb = qkvp.tile([128, PAIRS * N], BF16, tag="kt")    # [2h*64, p*2048+k]
        v_sb = qkvp.tile([128, RT, HEADS * 65], BF16, tag="v")  # [k, rt, h*65+d]
        ot = outp.tile([128, PAIRS * NQ], BF16, tag="ot")      # [inner, p*1024+q]
        wout_sb = outp.tile([128, KT * DIM], BF16, tag="wout")

        for k in range(KT):
            nc.sync.dma_start(wout_sb[:, k * DIM:(k + 1) * DIM],
                              wout_d[k * 128:(k + 1) * 128, :])

        with ExitStack() as proj:
            xp = proj.enter_context(tc.tile_pool(name="xp", bufs=2))
            xnp = proj.enter_context(tc.tile_pool(name="xnp", bufs=3))
            stat = proj.enter_context(tc.tile_pool(name="stat", bufs=4))
            xtp = proj.enter_context(tc.tile_pool(name="xtp", bufs=1))
            wp = proj.enter_context(tc.tile_pool(name="wp", bufs=1))
            pst = proj.enter_context(
                tc.tile_pool(name="pst", bufs=4, space=bass.MemorySpace.PSUM))
            psm = proj.enter_context(
                tc.tile_pool(name="psm", bufs=4, space=bass.MemorySpace.PSUM))

            xnt = xtp.tile([128, KT, N], BF16, tag="xnt")  # [dim, kt, row]
            wqkv_sb = wp.tile([128, KT * 3 * INNER], BF16, tag="wqkv")
            for k in range(KT):
                nc.sync.dma_start(
                    wqkv_sb[:, k * 3 * INNER:(k + 1) * 3 * INNER],
                    wqkv_d[k * 128:(k + 1) * 128, :])

            # ---- phase 1: LayerNorm + transpose into xnt ----
            for rt in range(RT):
                x_t = xp.tile([128, DIM], F32, tag="x")
                nc.sync.dma_start(x_t[:], x_d[rt * 128:(rt + 1) * 128, :])

                stats = stat.tile([128, 2, 6], F32, tag="bnst")
                xr = x_t[:].rearrange("p (s f) -> p s f", s=2)
                for s in range(2):
                    nc.vector.bn_stats(out=stats[:, s, :], in_=xr[:, s, :])
                mv = stat.tile([128, 2], F32, tag="bnag")
                nc.vector.bn_aggr(out=mv[:], in_=stats[:])
                rstd = stat.tile([128, 1], F32, tag="rstd")
                nc.scalar.activation(out=rstd[:], in_=mv[:, 1:2], func=AF.Sqrt,
                                     bias=eps_t[:], scale=1.0)
                nc.vector.reciprocal(out=rstd[:], in_=rstd[:])

                xn_t = xnp.tile([128, DIM], BF16, tag="xn")
                nc.vector.tensor_scalar(
                    out=xn_t[:], in0=x_t[:], scalar1=mv[:, 0:1],
                    scalar2=rstd[:], op0=mybir.AluOpType.subtract,
                    op1=mybir.AluOpType.mult)

                for k in range(KT):
                    ps = pst.tile([128, 128], BF16, tag="tr")
                    nc.tensor.transpose(ps[:], xn_t[:, k * 128:(k + 1) * 128],
                                        ident[:])
                    nc.vector.tensor_copy(
                        out=xnt[:, k, rt * 128:(rt + 1) * 128], in_=ps[:])

            # ---- phase 2: projections ----
            # Q^T (only my query rows = rolled rows 0:NQ)
            for p in range(PAIRS):
                for ch in range(NQ // 512):
                    ps = psm.tile([128, 512], F32, tag="mm")
                    for k in range(KT):
                        nc.tensor.matmul(
                            ps[:],
                            wqkv_sb[:, k * 3 * INNER + p * 128:
                                    k * 3 * INNER + (p + 1) * 128],
                            xnt[:, k, ch * 512:(ch + 1) * 512],
                            start=(k == 0), stop=(k == KT - 1))
                    nc.vector.tensor_copy(
                        out=qt[:, p * NQ + ch * 512: p * NQ + (ch + 1) * 512],
                        in_=ps[:])
            # K^T (all rows)
            for p in range(PAIRS):
                for ch in range(N // 512):
                    ps = psm.tile([128, 512], F32, tag="mm")
                    for k in range(KT):
                        nc.tensor.matmul(
                            ps[:],
                            wqkv_sb[:, k * 3 * INNER + INNER + p * 128:
                                    k * 3 * INNER + INNER + (p + 1) * 128],
                            xnt[:, k, ch * 512:(ch + 1) * 512],
                            start=(k == 0), stop=(k == KT - 1))
                    nc.vector.tensor_copy(
                        out=kt_sb[:, p * N + ch * 512: p * N + (ch + 1) * 512],
                        in_=ps[:])
            # V natural [rows, hd] (all rows), scattered into 65-stride blocks
            for rt in range(RT):
                for ch in range(2):
                    ps = psm.tile([128, 512], F32, tag="mm")
                    for k in range(KT):
                        nc.tensor.matmul(
                            ps[:],
                            xnt[:, k, rt * 128:(rt + 1) * 128],
                            wqkv_sb[:, k * 3 * INNER + 2 * INNER + ch * 512:
                                    k * 3 * INNER + 2 * INNER + (ch + 1) * 512],
                            start=(k == 0), stop=(k == KT - 1))
                    psv = ps[:].rearrange("p (h d) -> p h d", d=DH)
                    vdst = v_sb[:, rt, ch * 8 * 65:(ch * 8 + 8) * 65]
                    vdst = vdst.rearrange("p (h d) -> p h d", d=65)
                    nc.vector.tensor_copy(out=vdst[:, :, 0:DH], in_=psv[:])
            nc.gpsimd.memset(
                v_sb[:].rearrange("p r (h d) -> p r h d", d=65)[:, :, :, 64:65],
                1.0)

        # ---- phase 3: attention ----
        with ExitStack() as att:
            ap_pool = att.enter_context(tc.tile_pool(name="attn", bufs=2))
            sm_pool = att.enter_context(tc.tile_pool(name="smal", bufs=4))
            psd = att.enter_context(
                tc.tile_pool(name="psd", bufs=4, space=bass.MemorySpace.PSUM))
            pso = att.enter_context(
                tc.tile_pool(name="pso", bufs=2, space=bass.MemorySpace.PSUM))
            psr = att.enter_context(
                tc.tile_pool(name="psr", bufs=2, space=bass.MemorySpace.PSUM))

            for h in range(HEADS):
                p, hl = h // 2, (h % 2) * DH
                for ch in range(NQ // 512):
                    at = ap_pool.tile([128, RT, 512], BF16, tag="at")
                    for t in range(RT):
                        ps = psd.tile([128, 512], F32, tag="dots")
                        nc.tensor.matmul(
                            ps[:],
                            kt_sb[hl:hl + DH, p * N + t * 128:
                                  p * N + (t + 1) * 128],
                            qt[hl:hl + DH, p * NQ + ch * 512:
                               p * NQ + (ch + 1) * 512],
                            start=True, stop=True)
                        nc.scalar.activation(out=at[:, t, :], in_=ps[:],
                                             func=AF.Exp)
                    po = pso.tile([65, 512], F32, tag="ov")
                    for t in range(RT):
                        nc.tensor.matmul(
                            po[:], v_sb[:, t, h * 65:(h + 1) * 65],
                            at[:, t, :], start=(t == 0), stop=(t == RT - 1))
                    recip = sm_pool.tile([1, 512], F32, tag="recip")
                    nc.vector.reciprocal(out=recip[:], in_=po[64:65, :])
                    pr = psr.tile([64, 512], F32, tag="rep")
                    nc.tensor.matmul(pr[:], ones_col[:], recip[:],
                                     start=True, stop=True)
                    rep = sm_pool.tile([64, 512], F32, tag="repsb")
                    nc.vector.tensor_copy(out=rep[:], in_=pr[:])
                    nc.vector.tensor_mul(
                        out=ot[hl:hl + DH, p * NQ + ch * 512:
                               p * NQ + (ch + 1) * 512],
                        in0=po[0:DH, :], in1=rep[:])

        # ---- phase 4: output projection ----
        with ExitStack() as fin:
            orow = fin.enter_context(tc.tile_pool(name="orow", bufs=2))
            psf = fin.enter_context(
                tc.tile_pool(name="psf", bufs=4, space=bass.MemorySpace.PSUM))
            for m in range(NQ // 128):
                orow_t = orow.tile([128, DIM], F32, tag="orow")
                for ch in range(DIM // 512):
                    ps = psf.tile([128, 512], F32, tag="fmm")
                    for k in range(KT):
                        nc.tensor.matmul(
                            ps[:],
                            ot[:, k * NQ + m * 128:k * NQ + (m + 1) * 128],
                            wout_sb[:, k * DIM + ch * 512:
                                    k * DIM + (ch + 1) * 512],
                            start=(k == 0), stop=(k == KT - 1))
                    nc.vector.tensor_copy(
                        out=orow_t[:, ch * 512:(ch + 1) * 512], in_=ps[:])
                nc.sync.dma_start(out_d[m * 128:(m + 1) * 128, :], orow_t[:])


_NC_CACHE = None


def kernel(x, ln_gamma, ln_beta, w_qkv, w_out, b_out):
    global _NC_CACHE
    x = np.asarray(x, dtype=np.float32)
    ln_gamma = np.asarray(ln_gamma, dtype=np.float32)
    ln_beta = np.asarray(ln_beta, dtype=np.float32)
    w_qkv = np.asarray(w_qkv, dtype=np.float32)
    w_out = np.asarray(w_out, dtype=np.float32)
    b_out = np.asarray(b_out, dtype=np.float32)

    # fold gamma + softmax scale into w_qkv (host, exact f32)
    wqkv_eff = w_qkv * ln_gamma[:, None]
    wqkv_eff = wqkv_eff.copy()
    wqkv_eff[:, :INNER] *= SCALE
    qkv_bias = ln_beta @ w_qkv
    assert not np.any(qkv_bias), "nonzero ln_beta not supported on device"
    wqkv_bf = wqkv_eff.astype(ml_dtypes.bfloat16)
    wout_bf = w_out.astype(ml_dtypes.bfloat16)

    if _NC_CACHE is None:
        _NC_CACHE = _build_graph()
    nc = _NC_CACHE

    # clear any wedged NRT state left by a previous process on the cores
    try:
        import ctypes
        import jax
        jax.devices()
        _lib = ctypes.CDLL("/opt/axon/libaxon_pjrt.so")
        if hasattr(_lib, "axon_reset"):
            _lib.axon_reset.restype = ctypes.c_int64
            _lib.axon_reset()
    except Exception:
        pass

    in_maps = []
    for core in range(N_CORES):
        b, half = core // 2, core % 2
        xb = x[b] if half == 0 else np.roll(x[b], -NQ, axis=0)
        in_maps.append({"x": np.ascontiguousarray(xb),
                        "wqkv": wqkv_bf, "wout": wout_bf})

    res = run_bass_kernel_spmd(nc, in_maps, core_ids=list(range(N_CORES)))

    out = np.empty((B, N, DIM), dtype=np.float32)
    for core in range(N_CORES):
        b, half = core // 2, core % 2
        out[b, half * NQ:(half + 1) * NQ, :] = res.results[core]["out"]
    out += b_out
    return out
